# revision 22
# baseline (speedup 1.0000x reference)
"""Trainium2 Bass kernel for nn_Diff_SSM_Block (hourglass + Mamba SSM).

Sharding: 8 cores = 4 batches x 2-way split of d_inner (2048 -> 1024/core).
Per-core layout is feature-major (features on SBUF partitions, tokens on the
free dim). The selective scan runs on VectorE tensor_tensor_scan with d on
partitions and (dsub, t) segments on the free dim; the 16 state dims are
handled one scan per n with zeroed-dA carry columns injecting cross-chunk
state. x_proj partials and out_proj partials are pair-AllReduced per chunk.
The residual is applied by a DRAM->DRAM copy of x plus a CCE-accumulate DMA
of the fusion output.
"""
import sys
sys.path.insert(0, '/opt/trn_rl_repo')

import numpy as np
import ml_dtypes

import concourse.bass as bass
import concourse.bacc as bacc
import concourse.mybir as mybir
import concourse.tile as tile
from concourse.bass_utils import run_bass_kernel_spmd

F32 = mybir.dt.float32
BF16 = mybir.dt.bfloat16
AF = mybir.ActivationFunctionType
OP = mybir.AluOpType
BF = ml_dtypes.bfloat16

B, L, H = 4, 4096, 1024
DI, N, KC, DR = 2048, 16, 4, 64
DH = DI // 2              # d_inner half per core = 1024
EPS = 1e-6
TC = 256                  # tokens per chunk
NCH = L // TC             # chunks
NDS = DH // 128           # 8 d-subtiles per core
NHT = H // 128            # 8 feature tiles
SEG = TC + 1              # scan segment length (carry col + TC)

N_CORES = 8
PAIRS = [[0, 1], [2, 3], [4, 5], [6, 7]]


def _bf(a):
    return np.ascontiguousarray(a.astype(BF))


def _f32(a):
    return np.ascontiguousarray(a.astype(np.float32))


def shard_inputs(inp):
    """Build the 8 per-core input maps (host-side layout prep only)."""
    x = np.asarray(inp['x'], np.float32)
    in_proj_w = np.asarray(inp['in_proj_w'], np.float32)
    A = -np.exp(np.asarray(inp['A_log'], np.float32))      # (DI, N)
    assert np.allclose(A, A[0:1, :], rtol=1e-5), "A must be d-independent"
    a_scales = [float(A[0, n]) for n in range(N)]

    maps = []
    for c in range(N_CORES):
        b, hf = c // 2, c % 2
        lo, hi = hf * DH, (hf + 1) * DH
        xt = _f32(x[b].T)
        m = {
            'xT': xt,                                       # (H, L) f32
            'xTb': _bf(xt),                                 # (H, L) bf16
            # lhsT weights, bf16.  matmul computes lhsT.T @ rhs, lhsT=(K, M)
            'w_hgd1': _bf(np.asarray(inp['hgd_w1']).T),     # (H, 256)
            'w_hgd2': _bf(np.asarray(inp['hgd_w2']).T),     # (256, H)
            'w_xin': _bf(in_proj_w[lo:hi].T),               # (H, DH)
            'w_z': _bf(in_proj_w[DI + lo:DI + hi].T),       # (H, DH)
            'w_xp': _bf(np.asarray(inp['x_proj_w'])[:, lo:hi].T),   # (DH, 96)
            'w_dtp': _bf(np.asarray(inp['dt_proj_w'])[lo:hi].T),    # (DR, DH)
            'w_mo': _bf((np.asarray(inp['hgf_wm'], np.float64)
                         @ np.asarray(inp['out_proj_w'], np.float64))[:, lo:hi].T
                        .astype(np.float32)),               # (DH, 256)
            'w_hgfm': _bf(np.asarray(inp['hgf_wm']).T),     # (H, 256)
            'w_hgfr': _bf(np.asarray(inp['hgf_wr']).T),     # (H, 256)
            'w_hgff': _bf(np.asarray(inp['hgf_wf']).T),     # (256, H)
            # per-partition param columns (f32), merged (128, nds*w) layouts
            'conv_w': _f32(np.asarray(inp['conv_w'])[lo:hi].reshape(NDS, 128, KC)
                           .transpose(1, 0, 2).reshape(128, NDS * KC)),
            'conv_b': _f32(np.asarray(inp['conv_b'])[lo:hi].reshape(NDS, 128)
                           .T.reshape(128, NDS)),
            'dt_b': _f32(np.asarray(inp['dt_proj_b'])[lo:hi].reshape(NDS, 128)
                         .T.reshape(128, NDS)),
            'd_col': _f32(np.asarray(inp['D'])[lo:hi].reshape(NDS, 128)
                          .T.reshape(128, NDS)),
            'b_hgd1': _f32(np.asarray(inp['hgd_b1']).reshape(2, 128).T.reshape(128, 2)),
            'b_hgd2': _f32(np.asarray(inp['hgd_b2']).reshape(NHT, 128).T.reshape(128, NHT)),
            'b_hgfm': _f32(np.asarray(inp['hgf_bm']).reshape(2, 128).T.reshape(128, 2)),
            'b_hgfr': _f32(np.asarray(inp['hgf_br']).reshape(2, 128).T.reshape(128, 2)),
            'b_hgff': _f32(np.asarray(inp['hgf_bf']).reshape(NHT, 128).T.reshape(128, NHT)),
        }
        maps.append(m)
    return maps, a_scales


def build(nc, a_scales, n_passes=1, ablate=()):
    """ablate: set of section names to skip (timing experiments only):
    'scan' (sec 11), 'cc' (collectives), 'mm' (in/out_proj matmuls),
    'front' (LN/hgd), 'post' (hgf)."""
    # ---- DRAM I/O ----
    xT = nc.dram_tensor("xT", [H, L], F32, kind="ExternalInput")
    xTb = nc.dram_tensor("xTb", [H, L], BF16, kind="ExternalInput")
    wt = {}
    for name, shape, dt in [
        ('w_hgd1', (H, 256), BF16), ('w_hgd2', (256, H), BF16),
        ('w_xin', (H, DH), BF16), ('w_z', (H, DH), BF16),
        ('w_xp', (DH, 96), BF16), ('w_dtp', (DR, DH), BF16),
        ('w_mo', (DH, 256), BF16),
        ('w_hgfm', (H, 256), BF16), ('w_hgfr', (H, 256), BF16),
        ('w_hgff', (256, H), BF16),
        ('conv_w', (128, NDS * KC), F32), ('conv_b', (128, NDS), F32),
        ('dt_b', (128, NDS), F32), ('d_col', (128, NDS), F32),
        ('b_hgd1', (128, 2), F32), ('b_hgd2', (128, NHT), F32),
        ('b_hgfm', (128, 2), F32), ('b_hgfr', (128, 2), F32),
        ('b_hgff', (128, NHT), F32),
    ]:
        wt[name] = nc.dram_tensor(name, list(shape), dt, kind="ExternalInput")
    out_d = nc.dram_tensor("out", [H, L], F32, kind="ExternalOutput")

    with tile.TileContext(nc) as tc:
        with tc.tile_pool(name="wp", bufs=1) as wp, \
             tc.tile_pool(name="p1", bufs=1) as p1, \
             tc.tile_pool(name="p2", bufs=2) as p2, \
             tc.tile_pool(name="ps", bufs=4, space="PSUM") as ps, \
             tc.tile_pool(name="ps1", bufs=1, space="PSUM") as ps1, \
             tc.tile_pool(name="dp", bufs=4, space="DRAM") as dp:

            # ---- resident weights in SBUF ----
            def load_w(name, K, M, dt=BF16):
                nkt = (K + 127) // 128
                t = wp.tile([128, nkt * M], dt, tag=name)
                src = wt[name][:, :]
                for kt in range(nkt):
                    pr = min(128, K - kt * 128)
                    nc.sync.dma_start(t[:pr, kt * M:kt * M + M],
                                      src[kt * 128:kt * 128 + pr, :])
                return t

            w_hgd1 = load_w('w_hgd1', H, 256)
            w_hgd2 = load_w('w_hgd2', 256, H)
            w_xin = load_w('w_xin', H, DH)
            w_z = load_w('w_z', H, DH)
            w_xp = load_w('w_xp', DH, 96)
            w_dtp = load_w('w_dtp', DR, DH)
            w_mo = load_w('w_mo', DH, 256)
            w_hgfm = load_w('w_hgfm', H, 256)
            w_hgfr = load_w('w_hgfr', H, 256)
            w_hgff = load_w('w_hgff', 256, H)
            cols = {}
            for name, w in [('conv_w', NDS * KC), ('conv_b', NDS), ('dt_b', NDS),
                            ('d_col', NDS), ('b_hgd1', 2), ('b_hgd2', NHT),
                            ('b_hgfm', 2), ('b_hgfr', 2), ('b_hgff', NHT)]:
                t = wp.tile([128, w], F32, tag=name)
                nc.sync.dma_start(t[:], wt[name][:, :])
                cols[name] = t

            eps_col = wp.tile([1, 1], F32, tag="eps_col")
            nc.vector.memset(eps_col[:], EPS)
            ones_col = wp.tile([128, 1], BF16, tag="ones_col")
            nc.vector.memset(ones_col[:], 1.0)
            ones_row = wp.tile([1, 128], BF16, tag="ones_row")
            nc.vector.memset(ones_row[:], 1.0)
            # carry store: h state at chunk boundary per (n, dsub), f32
            carry = wp.tile([128, N * NDS], F32, tag="carry")
            nc.vector.memset(carry[:], 0.0)

            def lw(t, K_of, kt, M, mt, mw=128):
                """lhsT slice for K-tile kt, M columns [mt*mw, +mw)."""
                pr = min(128, K_of - kt * 128)
                return t[:pr, kt * M + mt * mw: kt * M + mt * mw + mw]

            if n_passes == 0:
                for c in range(NCH):
                    nc.sync.dma_start(out_d[:, c * TC:(c + 1) * TC],
                                      xT[:, c * TC:(c + 1) * TC])
                return nc

            state = {'prev_xin': None}
            pair_state = {}

            def front(c):
                cL = c * TC
                F = {}
                # 1. load bf16 x chunk; copy f32 x chunk into out (residual base)
                xb = p2.tile([128, NHT * TC], BF16, tag="xb")
                for ht in range(NHT):
                    nc.sync.dma_start(xb[:, ht * TC:(ht + 1) * TC],
                                      xTb[ht * 128:(ht + 1) * 128, cL:cL + TC])
                nc.sync.dma_start(out_d[:, cL:cL + TC], xT[:, cL:cL + TC])

                # 2. squares for LN stats
                sq = p1.tile([128, NHT * TC], BF16, tag="sq")
                for ht in range(NHT):
                    s = slice(ht * TC, (ht + 1) * TC)
                    nc.scalar.activation(sq[:, s], xb[:, s], AF.Square)

                # 3. LN stats via PE column sums
                mu_ps = ps1.tile([1, TC], F32, tag="mu_ps")
                ms_ps = ps1.tile([1, TC], F32, tag="ms_ps")
                for ht in range(NHT):
                    s = slice(ht * TC, (ht + 1) * TC)
                    nc.tensor.matmul(mu_ps[:], ones_col[:], xb[:, s],
                                     start=(ht == 0), stop=(ht == NHT - 1))
                for ht in range(NHT):
                    s = slice(ht * TC, (ht + 1) * TC)
                    nc.tensor.matmul(ms_ps[:], ones_col[:], sq[:, s],
                                     start=(ht == 0), stop=(ht == NHT - 1))
                st = p1.tile([1, 4 * TC], F32, tag="stats")
                mu_s, ms_s, tmp, rstd = (st[:, i * TC:(i + 1) * TC] for i in range(4))
                nc.scalar.activation(mu_s, mu_ps[:], AF.Copy, scale=1.0 / H)
                nc.scalar.activation(ms_s, ms_ps[:], AF.Copy, scale=1.0 / H)
                nc.vector.tensor_tensor(tmp, mu_s, mu_s, op=OP.mult)
                nc.vector.tensor_tensor(tmp, ms_s, tmp, op=OP.subtract)
                # rstd = exp(-0.5*ln(var+eps)) keeps ACT in the exp/ln table set
                nc.scalar.activation(tmp, tmp, AF.Ln, bias=eps_col[:])
                nc.scalar.activation(rstd, tmp, AF.Exp, scale=-0.5)
                stb = p1.tile([1, 2 * TC], BF16, tag="stb")
                rstd_b, nmr_b = stb[:, 0:TC], stb[:, TC:2 * TC]
                nc.scalar.activation(rstd_b, rstd, AF.Copy)
                nmr_f = p1.tile([1, TC], F32, tag="nmr_f")
                nc.vector.tensor_tensor(nmr_f[:], mu_s, rstd, op=OP.mult)
                nc.scalar.activation(nmr_b, nmr_f[:], AF.Copy, scale=-1.0)
                # broadcast to 128 partitions via K=1 matmul
                rb_ps = ps.tile([128, TC], F32, tag="mm")
                nc.tensor.matmul(rb_ps[:], ones_row[:], rstd_b, start=True, stop=True)
                rstd_bc = p1.tile([128, TC], BF16, tag="rstd_bc")
                nc.scalar.activation(rstd_bc[:], rb_ps[:], AF.Copy)
                nb_ps = ps.tile([128, TC], F32, tag="mm")
                nc.tensor.matmul(nb_ps[:], ones_row[:], nmr_b, start=True, stop=True)
                nmr_bc = p1.tile([128, TC], BF16, tag="nmr_bc")
                nc.scalar.activation(nmr_bc[:], nb_ps[:], AF.Copy)

                # 4. x1 = xb*rstd + (-mu*rstd)
                x1 = p2.tile([128, NHT * TC], BF16, tag="x1")
                for ht in range(NHT):
                    s = slice(ht * TC, (ht + 1) * TC)
                    nc.vector.tensor_tensor(x1[:, s], xb[:, s], rstd_bc[:], op=OP.mult)
                    nc.vector.tensor_tensor(x1[:, s], x1[:, s], nmr_bc[:], op=OP.add)

                # 5. hourglass dense
                hd1 = p1.tile([128, 2 * TC], BF16, tag="hd1")
                for mt in range(2):
                    p = ps.tile([128, TC], F32, tag="mm")
                    for kt in range(NHT):
                        nc.tensor.matmul(p[:], lw(w_hgd1, H, kt, 256, mt),
                                         x1[:, kt * TC:(kt + 1) * TC],
                                         start=(kt == 0), stop=(kt == NHT - 1))
                    nc.scalar.activation(hd1[:, mt * TC:(mt + 1) * TC], p[:],
                                         AF.Silu, bias=cols['b_hgd1'][:, mt:mt + 1])
                hd = p1.tile([128, NHT * TC], BF16, tag="hd")
                for mt in range(NHT):
                    p = ps.tile([128, TC], F32, tag="mm")
                    for kt in range(2):
                        nc.tensor.matmul(p[:], lw(w_hgd2, 256, kt, H, mt),
                                         hd1[:, kt * TC:(kt + 1) * TC],
                                         start=(kt == 0), stop=(kt == 1))
                    nc.scalar.activation(hd[:, mt * TC:(mt + 1) * TC], p[:],
                                         AF.Identity, bias=cols['b_hgd2'][:, mt:mt + 1])

                # 6. in_proj -> x_in (into conv-extended segs) and z -> silu(z)
                xin = p2.tile([128, NDS * (TC + 3)], BF16, tag="xin")
                zs = p2.tile([128, NDS * TC], BF16, tag="zs")
                for mt in range(NDS):
                    p = ps.tile([128, TC], F32, tag="mm")
                    for kt in range(NHT):
                        nc.tensor.matmul(p[:], lw(w_xin, H, kt, DH, mt),
                                         hd[:, kt * TC:(kt + 1) * TC],
                                         start=(kt == 0), stop=(kt == NHT - 1))
                    nc.scalar.activation(
                        xin[:, mt * (TC + 3) + 3: (mt + 1) * (TC + 3)], p[:], AF.Copy)
                    p2m = ps.tile([128, TC], F32, tag="mm")
                    for kt in range(NHT):
                        nc.tensor.matmul(p2m[:], lw(w_z, H, kt, DH, mt),
                                         hd[:, kt * TC:(kt + 1) * TC],
                                         start=(kt == 0), stop=(kt == NHT - 1))
                    nc.scalar.activation(zs[:, mt * TC:(mt + 1) * TC], p2m[:], AF.Silu)

                # 7. conv halo fill + causal conv + silu
                halo_dst = xin[:].rearrange("p (s q) -> p s q", q=TC + 3)[:, :, 0:3]
                if state['prev_xin'] is None:
                    nc.vector.memset(halo_dst, 0.0)
                else:
                    halo_src = state['prev_xin'][:].rearrange(
                        "p (s q) -> p s q", q=TC + 3)[:, :, TC:TC + 3]
                    nc.vector.tensor_copy(halo_dst, halo_src)
                state['prev_xin'] = xin
                xc = p2.tile([128, NDS * TC], BF16, tag="xc")
                cvt = p1.tile([128, TC], BF16, tag="cvt")
                for ds in range(NDS):
                    base = ds * (TC + 3)
                    so = slice(ds * TC, (ds + 1) * TC)
                    nc.vector.tensor_scalar(
                        out=xc[:, so], in0=xin[:, base:base + TC],
                        scalar1=cols['conv_w'][:, ds * KC:ds * KC + 1],
                        scalar2=None, op0=OP.mult)
                    for k in range(1, KC):
                        nc.vector.tensor_scalar(
                            out=cvt[:], in0=xin[:, base + k:base + k + TC],
                            scalar1=cols['conv_w'][:, ds * KC + k:ds * KC + k + 1],
                            scalar2=None, op0=OP.mult)
                        nc.vector.tensor_tensor(xc[:, so], xc[:, so], cvt[:], op=OP.add)
                    nc.scalar.activation(xc[:, so], xc[:, so], AF.Silu,
                                         bias=cols['conv_b'][:, ds:ds + 1])

                # 8. x_dbl partial -> pair AllReduce (bf16)
                xd_ps = ps1.tile([96, TC], F32, tag="xd")
                for kt in range(NDS):
                    nc.tensor.matmul(xd_ps[:], lw(w_xp, DH, kt, 96, 0, 96),
                                     xc[:, kt * TC:(kt + 1) * TC],
                                     start=(kt == 0), stop=(kt == NDS - 1))
                xd_sb = p1.tile([96, TC], BF16, tag="xd_sb")
                nc.scalar.activation(xd_sb[:], xd_ps[:], AF.Copy)
                cc_in = dp.tile([96, TC], BF16, tag="cc_in")
                cc_out = dp.tile([96, TC], BF16, tag="cc_out")
                nc.sync.dma_start(cc_in[:], xd_sb[:])
                if 'cc' not in ablate:
                    nc.gpsimd.collective_compute(
                        "AllReduce", OP.add, replica_groups=PAIRS,
                        ins=[cc_in.opt()], outs=[cc_out.opt()])
                else:
                    nc.sync.dma_start(cc_out[:], cc_in[:])
                dtlo = p2.tile([DR, TC], BF16, tag="dtlo")
                nc.sync.dma_start(dtlo[:], cc_out[0:DR, :])
                bcb = p2.tile([128, 2 * N * TC], BF16, tag="bcb")  # B then C segs
                for g in range(4):
                    src = cc_out[DR + g * 8: DR + (g + 1) * 8, :].partition_broadcast(128)
                    dst = bcb[:, g * 8 * TC:(g + 1) * 8 * TC].rearrange(
                        "p (n t) -> p n t", t=TC)
                    nc.sync.dma_start(dst, src)
                # hgf r-branch (only needs x1) computed in front
                r_s = p2.tile([128, 2 * TC], BF16, tag="r_s", bufs=6)
                for mt in range(2):
                    p = ps.tile([128, TC], F32, tag="mm")
                    for kt in range(NHT):
                        nc.tensor.matmul(p[:], lw(w_hgfr, H, kt, 256, mt),
                                         x1[:, kt * TC:(kt + 1) * TC],
                                         start=(kt == 0), stop=(kt == NHT - 1))
                    nc.scalar.activation(r_s[:, mt * TC:(mt + 1) * TC], p[:],
                                         AF.Silu, bias=cols['b_hgfr'][:, mt:mt + 1])
                F.update(zs=zs, xc=xc, dtlo=dtlo, bcb=bcb, r_s=r_s)
                return F

            def back1(c, F):
                zs, xc, dtlo, bcb = F['zs'], F['xc'], F['dtlo'], F['bcb']

                # 9. dt = softplus(dt_proj @ dtlo + dt_b): Exp into segs, Ln in place
                dt_m = p1.tile([128, NDS * TC], BF16, tag="dt_m")
                for mt in range(NDS):
                    p = ps.tile([128, TC], F32, tag="mm")
                    nc.tensor.matmul(p[:], lw(w_dtp, DR, 0, DH, mt), dtlo[:],
                                     start=True, stop=True)
                    nc.scalar.activation(dt_m[:, mt * TC:(mt + 1) * TC], p[:],
                                         AF.Exp, bias=cols['dt_b'][:, mt:mt + 1])
                nc.scalar.activation(dt_m[:], dt_m[:], AF.Ln, bias=1.0)

                # 10. dtu = dt * x_c
                dtu = p1.tile([128, NDS * TC], BF16, tag="dtu")
                nc.vector.tensor_tensor(dtu[:], dt_m[:], xc[:], op=OP.mult)

                # 11. scan per n (in-place on the b buffer)
                yac = p1.tile([128, NDS * TC], BF16, tag="yac")
                dt_seg = dt_m[:].rearrange("p (s t) -> p s t", t=TC)
                dtu_seg = dtu[:].rearrange("p (s t) -> p s t", t=TC)
                yv = yac[:].rearrange("p (s t) -> p s t", t=TC)
                if 'scan' in ablate:
                    nc.vector.tensor_copy(yac[:], dtu[:])
                for n in range(N if 'scan' not in ablate else 0):
                    dA = p2.tile([128, NDS * SEG], BF16, tag="dA")
                    bb = p2.tile([128, NDS * SEG], BF16, tag="bb")
                    dA_seg = dA[:].rearrange("p (s q) -> p s q", q=SEG)
                    bb_seg = bb[:].rearrange("p (s q) -> p s q", q=SEG)
                    nc.vector.memset(dA_seg[:, :, 0:1], 0.0)
                    nc.scalar.activation(dA_seg[:, :, 1:SEG], dt_seg, AF.Exp,
                                         scale=a_scales[n])
                    bn = bcb[:, n * TC:(n + 1) * TC]
                    bn_rep = bn.unsqueeze(1).broadcast_to((128, NDS, TC))
                    nc.vector.tensor_tensor(bb_seg[:, :, 1:SEG], dtu_seg, bn_rep,
                                            op=OP.mult)
                    # inject carries (f32 -> bf16 copy)
                    nc.vector.tensor_copy(bb_seg[:, :, 0:1],
                                          carry[:, n * NDS:(n + 1) * NDS].unsqueeze(2))
                    nc.vector.tensor_tensor_scan(bb[:], dA[:], bb[:], 0.0,
                                                 op0=OP.mult, op1=OP.add)
                    # save carries for next chunk
                    nc.vector.tensor_copy(carry[:, n * NDS:(n + 1) * NDS].unsqueeze(2),
                                          bb_seg[:, :, SEG - 1:SEG])
                    cn = bcb[:, (N + n) * TC:(N + n + 1) * TC]
                    cn_rep = cn.unsqueeze(1).broadcast_to((128, NDS, TC))
                    nc.vector.tensor_tensor(bb_seg[:, :, 1:SEG], bb_seg[:, :, 1:SEG],
                                            cn_rep, op=OP.mult)
                    if n == 0:
                        nc.vector.tensor_copy(yv, bb_seg[:, :, 1:SEG])
                    else:
                        nc.vector.tensor_tensor(yv, yv, bb_seg[:, :, 1:SEG], op=OP.add)

                # 12. gate: yg = (yac + xc*D) * zs
                dterm = p1.tile([128, TC], BF16, tag="dterm")
                for ds in range(NDS):
                    so = slice(ds * TC, (ds + 1) * TC)
                    nc.vector.tensor_scalar(
                        out=dterm[:], in0=xc[:, so],
                        scalar1=cols['d_col'][:, ds:ds + 1], scalar2=None, op0=OP.mult)
                    nc.vector.tensor_tensor(yac[:, so], yac[:, so], dterm[:], op=OP.add)
                nc.vector.tensor_tensor(yac[:], yac[:], zs[:], op=OP.mult)

                # 13. fused (hgf_wm @ out_proj) partial -> pair AllReduce (bf16)
                ostg = p1.tile([128, 2 * TC], BF16, tag="ostg")
                for mt in range(2):
                    p = ps.tile([128, TC], F32, tag="mm")
                    for kt in range(NDS):
                        nc.tensor.matmul(p[:], lw(w_mo, DH, kt, 256, mt),
                                         yac[:, kt * TC:(kt + 1) * TC],
                                         start=(kt == 0), stop=(kt == NDS - 1))
                    nc.scalar.activation(ostg[:, mt * TC:(mt + 1) * TC], p[:], AF.Copy)
                if c % 2 == 0:
                    ci_t = dp.tile([256, 2 * TC], BF16, tag="cc2_in")
                    pair_state['cc2_in'] = ci_t
                ci = pair_state['cc2_in']
                half = c % 2
                nc.sync.dma_start(
                    ci[:, half * TC:(half + 1) * TC].rearrange("(s p) t -> p s t", p=128),
                    ostg[:].rearrange("p (s t) -> p s t", t=TC))
                return {'r_s': F['r_s']}

            def back2(g, rs_pair):
                # pair g covers chunks 2g, 2g+1
                ci = pair_state['cc2_in_run']
                co = dp.tile([256, 2 * TC], BF16, tag="cc2_out")
                if 'cc' not in ablate:
                    nc.gpsimd.collective_compute(
                        "AllReduce", OP.add, replica_groups=PAIRS,
                        ins=[ci.opt()], outs=[co.opt()])
                else:
                    nc.sync.dma_start(co[:], ci[:])
                for sub in range(2):
                    c = 2 * g + sub
                    mpre = p1.tile([128, 2 * TC], BF16, tag="x12")
                    nc.sync.dma_start(
                        mpre[:].rearrange("p (s t) -> p s t", t=TC),
                        co[:, sub * TC:(sub + 1) * TC].rearrange("(s p) t -> p s t", p=128))
                    cL = c * TC
                    r_s = rs_pair[sub]
                    # hourglass fusion m branch (matmul pre-folded on host)
                    mrs = p1.tile([128, 2 * TC], BF16, tag="mrs")
                    for mt in range(2):
                        nc.scalar.activation(mrs[:, mt * TC:(mt + 1) * TC],
                                             mpre[:, mt * TC:(mt + 1) * TC],
                                             AF.Silu, bias=cols['b_hgfm'][:, mt:mt + 1])
                    nc.vector.tensor_tensor(mrs[:], mrs[:], r_s[:], op=OP.mult)
                    for half in range(2):
                        fuse = p1.tile([128, 4 * TC], F32, tag="fuse")
                        for mi in range(4):
                            mt = half * 4 + mi
                            p = ps.tile([128, TC], F32, tag="mm")
                            for kt in range(2):
                                nc.tensor.matmul(p[:], lw(w_hgff, 256, kt, H, mt),
                                                 mrs[:, kt * TC:(kt + 1) * TC],
                                                 start=(kt == 0), stop=(kt == 1))
                            nc.scalar.activation(fuse[:, mi * TC:(mi + 1) * TC], p[:],
                                                 AF.Identity,
                                                 bias=cols['b_hgff'][:, mt:mt + 1])
                        ov = out_d[half * 512:(half + 1) * 512, cL:cL + TC].rearrange(
                            "(s p) t -> p s t", p=128)
                        nc.gpsimd.dma_start(
                            ov, fuse[:].rearrange("p (s t) -> p s t", t=TC),
                            accum_op=OP.add)

            for _pass in range(n_passes):
                fronts = {0: front(0)}
                rs_buf = {}
                pending = []
                for c in range(NCH):
                    if c + 1 < NCH:
                        fronts[c + 1] = front(c + 1)
                    F = fronts.pop(c)
                    rs_buf[c] = back1(c, F)['r_s']
                    if c % 2 == 1:
                        pending.append((c // 2, (rs_buf.pop(c - 1), rs_buf.pop(c)),
                                        pair_state['cc2_in']))
                    if len(pending) > 2:
                        g, rsp, ci_t = pending.pop(0)
                        pair_state['cc2_in_run'] = ci_t
                        back2(g, rsp)
                while pending:
                    g, rsp, ci_t = pending.pop(0)
                    pair_state['cc2_in_run'] = ci_t
                    back2(g, rsp)
    return nc


_CACHE = {}
LAST_RESULT = None


def kernel(**inputs):
    global LAST_RESULT
    maps, a_scales = shard_inputs(inputs)
    key = 'prog'
    if key not in _CACHE:
        nc = bacc.Bacc('TRN2', target_bir_lowering=False,
                       num_devices=N_CORES, num_swdge_queues=4)
        build(nc, a_scales)
        nc.finalize()
        _CACHE[key] = nc
    nc = _CACHE[key]
    res = run_bass_kernel_spmd(nc, maps, core_ids=list(range(N_CORES)))
    LAST_RESULT = res
    out = np.empty((B, L, H), np.float32)
    for b in range(B):
        out[b] = res.results[2 * b]['out'].T
    return out
